# revision 34
# baseline (speedup 1.0000x reference)
"""MultiHeadAttention Trainium2 kernel (8 NeuronCores, Bass/Tile).

Problem: B=2, S=2048, D=1024, H=16, DK=64 fp32 MHA (torch-Linear style
projections, softmax attention, output projection).

Sharding: core c = (batch b = c//4, head-group g = c%4); each core handles
4 heads of one batch in a transposed layout (features on partitions,
sequence on the free axis).

Schedule: the Scalar engine's EXP stream (128 tiles x ~1.0us) is the
critical resource; everything else is arranged to keep it saturated:
  prefix : DMA w + xk/xq first column-block; project kh(m0,n0)+qh(m0,n0)
           only, so the first scores matmul fires ~3MB into the input
           stream instead of after all of xk.
  rounds : 8 rounds (hp-major), one per (qt, hp). Round r emits, per kt
           slot: scores matmul pair (PE-tile-packed K=64 halves run
           concurrently), ACT exp, the PV chain of round r-1 paced evenly,
           and background PE units on an explicit per-slot schedule:
           remaining kh/qh projection column-blocks (each feeding scores
           1-4 rounds ahead), v-projection (feeding round-1 PV), and
           oproj of finished qt.  Emission order IS dataflow, so every
           unit is placed before its first consumer.
  tail   : PV of the last round interleaved with oproj(2), then oproj(3)
           with PSUM->SBUF copies alternating ACT/DVE.
Softmax denominators come free via a ones column in the PV stationary
(so Wo@bv folds into a host-side constant). No collectives; host sums 4
fp16 partials per batch.
"""

import numpy as np

B, S, D, H = 2, 2048, 1024, 16
DK = D // H          # 64
N_CORES = 8
HG = H // 4          # 4 head-groups
HL = 4               # heads per core
FEAT = HL * DK       # 256 per-core features
NQT = S // 512       # 4 query tiles
NKT = S // 128       # 16 key tiles
NDT = D // 128       # 8 contraction tiles (d-model)

_cache = {}


def _build():
    import concourse.mybir as mybir
    import concourse.tile as tile
    from concourse import bacc

    fp32 = mybir.dt.float32
    fp16 = mybir.dt.float16
    bf16 = mybir.dt.bfloat16

    nc = bacc.Bacc("TRN2", target_bir_lowering=False, debug=False,
                   num_devices=N_CORES)

    # DRAM inputs, host-prearranged so every DMA row is >=2KB contiguous
    xk_d = nc.dram_tensor("xk_d", [128, NQT, NDT, 512], fp16,
                          kind="ExternalInput").ap()
    xq_d = nc.dram_tensor("xq_d", [128, NQT, NDT, 512], fp16,
                          kind="ExternalInput").ap()
    xv_d = nc.dram_tensor("xv_d", [128, NKT, NDT, 128], fp16,
                          kind="ExternalInput").ap()
    wq_d = nc.dram_tensor("wq_d", [128, 2, NDT, 128], fp16,
                          kind="ExternalInput").ap()
    wk_d = nc.dram_tensor("wk_d", [128, 2, NDT, 128], fp16,
                          kind="ExternalInput").ap()
    wv_d = nc.dram_tensor("wv_d", [128, NDT, FEAT], fp16, kind="ExternalInput").ap()
    wo_d = nc.dram_tensor("wo_d", [128, 2, D], fp16, kind="ExternalInput").ap()
    bq_d = nc.dram_tensor("bq_d", [128, 2, 1], fp32, kind="ExternalInput").ap()
    bk_d = nc.dram_tensor("bk_d", [128, 2, 1], fp32, kind="ExternalInput").ap()
    out_d = nc.dram_tensor("partialT", [D, S], fp16, kind="ExternalOutput").ap()

    with tile.TileContext(nc) as tc:
        with (
            tc.tile_pool(name="xin", bufs=1) as xin,
            tc.tile_pool(name="win", bufs=1) as win,
            tc.tile_pool(name="proj", bufs=1) as proj,
        ):
            # ---- DMA emission order is the prefetch schedule ----
            wk3 = win.tile([128, 2, NDT, 128], fp16, tag="wk")
            wq3 = win.tile([128, 2, NDT, 128], fp16, tag="wq")
            wv3 = win.tile([128, NDT, FEAT], fp16, tag="wv")
            wo3 = win.tile([128, 2, D], fp16, tag="wo")
            bq3 = win.tile([128, 2, 1], fp32, tag="bq")
            bk3 = win.tile([128, 2, 1], fp32, tag="bk")
            xk3 = xin.tile([128, NQT, NDT, 512], fp16, tag="xk")
            xq3 = xin.tile([128, NQT, NDT, 512], fp16, tag="xq")
            xv3 = xin.tile([128, NKT, NDT, 128], fp16, tag="xv")

            # single engine issues inputs: the stream must drain strictly
            # in prefetch-priority order (two engines pull concurrently
            # and starve the critical earliest chunks)
            nc.sync.dma_start(wk3[:, 0], wk_d[:, 0])
            nc.sync.dma_start(wq3[:, 0], wq_d[:, 0])
            nc.sync.dma_start(bk3[:], bk_d)
            nc.sync.dma_start(bq3[:], bq_d)
            nc.sync.dma_start(xk3[:, 0], xk_d[:, 0])
            nc.sync.dma_start(xq3[:, 0], xq_d[:, 0])
            nc.sync.dma_start(xk3[:, 1], xk_d[:, 1])
            nc.sync.dma_start(xq3[:, 1], xq_d[:, 1])
            nc.sync.dma_start(xk3[:, 2], xk_d[:, 2])
            nc.sync.dma_start(xk3[:, 3], xk_d[:, 3])
            nc.sync.dma_start(wk3[:, 1], wk_d[:, 1])
            nc.sync.dma_start(wq3[:, 1], wq_d[:, 1])
            nc.sync.dma_start(wv3[:], wv_d)
            for c in range(4):
                nc.sync.dma_start(xv3[:, c * 4:(c + 1) * 4],
                                  xv_d[:, c * 4:(c + 1) * 4])
            nc.sync.dma_start(wo3[:], wo_d)
            nc.sync.dma_start(xq3[:, 2], xq_d[:, 2])
            nc.sync.dma_start(xq3[:, 3], xq_d[:, 3])

            # ---- persistent intermediates ----
            qh3 = proj.tile([128, 2, S], fp16, tag="qh")   # pair-packed
            kh3 = proj.tile([128, 2, S], fp16, tag="kh")
            vha = proj.tile([128, NKT, HL, DK + 1], bf16, tag="vha")
            ot3 = proj.tile([128, 2, S], fp16, tag="outT")

            # ---- prefix: kh(m0,n0) + qh(m0,n0) only ----
            with tc.tile_pool(name="pskh", bufs=1, space="PSUM") as pskh:
                kacc = pskh.tile([128, 512], fp32, tag="kacc")
                for kt in range(NDT):
                    nc.tensor.matmul(
                        kacc[:], wk3[:, 0, kt, :], xk3[:, 0, kt, :],
                        start=(kt == 0), stop=(kt == NDT - 1))
                nc.vector.tensor_scalar_add(
                    kh3[:, 0, 0:512], kacc[:], bk3[:, 0, :])
                qacc = pskh.tile([128, 512], fp32, tag="qacc")
                for kt in range(NDT):
                    nc.tensor.matmul(
                        qacc[:], wq3[:, 0, kt, :], xq3[:, 0, kt, :],
                        start=(kt == 0), stop=(kt == NDT - 1))
                nc.vector.tensor_scalar_add(
                    qh3[:, 0, 0:512], qacc[:], bq3[:, 0, :])

            with (
                tc.tile_pool(name="pexp", bufs=17) as pexp,
                tc.tile_pool(name="pout", bufs=2) as pout,
                tc.tile_pool(name="pnrm", bufs=2) as pnrm,
                tc.tile_pool(name="pp", bufs=2, space="PSUM") as pp,
                tc.tile_pool(name="ps2", bufs=2, space="PSUM") as ps2,
                tc.tile_pool(name="pspv", bufs=2, space="PSUM") as pspv,
            ):
                nc.gpsimd.memset(vha[:, :, :, DK], 1.0)  # ones column

                # ---- background work units (~0.4-0.9us of PE each) ----
                # Units allocate their PSUM tile lazily (inside the first
                # closure) so pool slot rotation follows emission order; a
                # unit's halves are adjacent so at most 2 accumulation
                # chains are open per pp slot pair.
                _acc = {}

                def proj_piece(key, w3, b3, x3, dst, m, n, piece):
                    if piece == 0:
                        _acc[(key, m, n)] = pp.tile(
                            [128, 512], fp32, tag="acc",
                            name=f"pa_{key}{m}{n}")
                    ps = _acc[(key, m, n)]
                    for kt in range(piece * 2, piece * 2 + 2):
                        nc.tensor.matmul(
                            ps[:], w3[:, m, kt, :], x3[:, n, kt, :],
                            start=(kt == 0), stop=(kt == NDT - 1))
                    if piece == 3:
                        nc.vector.tensor_scalar_add(
                            dst[:, m, n * 512:(n + 1) * 512], ps[:],
                            b3[:, m, :])

                def qh_unit(m, n, slots):
                    return [(slots[p], lambda p=p: proj_piece(
                                "q", wq3, bq3, xq3, qh3, m, n, p))
                            for p in range(4)]

                def kh_unit(m, n, slots):
                    return [(slots[p], lambda p=p: proj_piece(
                                "k", wk3, bk3, xk3, kh3, m, n, p))
                            for p in range(4)]

                def v_piece(st, piece):
                    if piece == 0:
                        _acc[("v", st)] = pp.tile(
                            [128, 256], fp32, tag="acc", name=f"vacc{st}")
                    ps = _acc[("v", st)]
                    for kt in range(piece * 2, piece * 2 + 2):
                        nc.tensor.matmul(
                            ps[:], xv3[:, st, kt, :], wv3[:, kt, :],
                            start=(kt == 0), stop=(kt == NDT - 1))
                    if piece == 3:
                        nc.vector.tensor_copy(vha[:, st, :, 0:DK], ps[:])

                def v_unit(st, slots):
                    return [(slots[p], lambda p=p: v_piece(st, p))
                            for p in range(4)]

                _po3 = {}

                def oproj_unit(qt, jt, on_act=False, ps_pool=None,
                               dma_eng=None):
                    if jt == 0:
                        _po3[qt] = pout.tile([128, NDT, 512], fp16,
                                             tag="po", name=f"po{qt}")
                    pool = ps_pool if ps_pool is not None else pp
                    tag = "pv" if pool is pspv else "acc"
                    ps = pool.tile([128, 512], fp32, tag=tag,
                                   name=f"ops{qt}{jt}")
                    for m in range(2):
                        nc.tensor.matmul(
                            ps[:], wo3[:, m, jt * 128:(jt + 1) * 128],
                            ot3[:, m, qt * 512:(qt + 1) * 512],
                            start=(m == 0), stop=(m == 1))
                    if on_act:   # tail: ACT is idle, DVE is busier
                        nc.scalar.copy(_po3[qt][:, jt, :], ps[:])
                    else:
                        nc.vector.tensor_copy(_po3[qt][:, jt, :], ps[:])
                    eng = dma_eng if dma_eng is not None else nc.sync
                    eng.dma_start(
                        out_d[jt * 128:(jt + 1) * 128,
                              qt * 512:(qt + 1) * 512], _po3[qt][:, jt, :])

                def oproj_units(qt, slots):
                    return [(s, lambda jt=jt: oproj_unit(qt, jt))
                            for jt, s in enumerate(slots)]

                # ---- round machinery ----
                def scores_kt(qt, hp, kt):
                    s2 = ps2.tile([128, 1024], fp32, tag="s2")
                    nc.tensor.matmul(
                        s2[:, 0:512],
                        kh3[0:64, hp, kt * 128:(kt + 1) * 128],
                        qh3[0:64, hp, qt * 512:(qt + 1) * 512],
                        start=True, stop=True)
                    nc.tensor.matmul(
                        s2[:, 512:1024],
                        kh3[64:128, hp, kt * 128:(kt + 1) * 128],
                        qh3[64:128, hp, qt * 512:(qt + 1) * 512],
                        start=True, stop=True)
                    e2 = pexp.tile([128, 1024], bf16, tag="e2")
                    nc.scalar.activation(
                        e2[:], s2[:],
                        mybir.ActivationFunctionType.Exp, scale=0.125)
                    return e2

                def pv_kt(hp, e2s, kt, pva, pvb):
                    nc.tensor.matmul(
                        pva[:], vha[:, kt, 2 * hp, :], e2s[kt][:, 0:512],
                        start=(kt == 0), stop=(kt == NKT - 1))
                    nc.tensor.matmul(
                        pvb[:], vha[:, kt, 2 * hp + 1, :],
                        e2s[kt][:, 512:1024],
                        start=(kt == 0), stop=(kt == NKT - 1))

                def pv_norm(qt, hp, pva, pvb):
                    for pv, half in ((pva, 0), (pvb, 1)):
                        srow = pnrm.tile([1, 512], fp32, tag="srow")
                        nc.vector.tensor_copy(srow[:], pv[DK:DK + 1, :])
                        inv = pnrm.tile([1, 512], fp32, tag="inv")
                        nc.vector.reciprocal_approx_fast(inv[:], srow[:])
                        invb = pnrm.tile([64, 512], fp32, tag="invb")
                        nc.gpsimd.partition_broadcast(invb[:], inv[:])
                        nc.vector.tensor_tensor(
                            ot3[half * 64:(half + 1) * 64, hp,
                                qt * 512:(qt + 1) * 512],
                            pv[0:DK, :], invb[:], mybir.AluOpType.mult)

                def emit_round(qt, hp, prev, bg):
                    """One exp-bound round.  Per kt slot: scores+exp, then
                    background units whose scheduled slot has arrived (in
                    list order — emission order IS dataflow), then the PV
                    chain of `prev` paced ~9/8 kt per slot with its norm
                    right after the last chain matmul."""
                    e2s = []
                    pv_done = 0
                    if prev is not None:
                        pqt, php, pe2s = prev
                        pva = pspv.tile([DK + 1, 512], fp32, tag="pv")
                        pvb = pspv.tile([DK + 1, 512], fp32, tag="pv")
                    for kt in range(NKT):
                        e2s.append(scores_kt(qt, hp, kt))
                        while _bg_done[0] < len(bg) and \
                                bg[_bg_done[0]][0] <= kt:
                            bg[_bg_done[0]][1]()
                            _bg_done[0] += 1
                        if prev is not None:
                            pv_want = min(NKT + 1, ((kt + 1) * 5) // 4)
                            while pv_done < pv_want:
                                if pv_done < NKT:
                                    pv_kt(php, pe2s, pv_done, pva, pvb)
                                else:
                                    pv_norm(pqt, php, pva, pvb)
                                pv_done += 1
                    while _bg_done[0] < len(bg):   # drain leftovers
                        bg[_bg_done[0]][1]()
                        _bg_done[0] += 1
                    if prev is not None and pv_done <= NKT:
                        while pv_done < NKT:
                            pv_kt(php, pe2s, pv_done, pva, pvb)
                            pv_done += 1
                        pv_norm(pqt, php, pva, pvb)
                    _bg_done[0] = 0
                    return e2s

                # ---- rounds, hp-major ----
                # Background placement is deadline-driven:
                #   kh(0,n) before scores kt=4n of the SAME round 0;
                #   qh(m,n) one round before scores(qt=n, hp=m);
                #   kh(1,n) any time before round 4;
                #   v(st) before PV(0,0) kt=st in round 1;
                #   oproj(qt) after pv_norm(qt, 1).
                ROUNDS = [(0, 0), (1, 0), (2, 0), (3, 0),
                          (0, 1), (1, 1), (2, 1), (3, 1)]
                BG = {
                    0: kh_unit(0, 1, (2, 2, 3, 3))
                       + qh_unit(0, 1, (4, 4, 5, 5))
                       + kh_unit(0, 2, (6, 6, 7, 7))
                       + kh_unit(0, 3, (8, 8, 9, 9))
                       + [u for j in range(6)
                          for u in v_unit(j, (10 + j,) * 4)],
                    1: [u for j in range(6, NKT)
                        for u in v_unit(j, (j - 6,) * 4)]
                       + qh_unit(0, 2, (11, 11, 12, 12)),
                    2: qh_unit(0, 3, (0, 1, 2, 3))
                       + kh_unit(1, 0, (4, 5, 6, 7))
                       + kh_unit(1, 1, (8, 9, 10, 11)),
                    3: qh_unit(1, 0, (0, 1, 2, 3))
                       + kh_unit(1, 2, (4, 5, 6, 7))
                       + kh_unit(1, 3, (8, 9, 10, 11)),
                    4: qh_unit(1, 1, (0, 1, 2, 3)),
                    5: qh_unit(1, 2, (0, 1, 2, 3)),
                    6: qh_unit(1, 3, (0, 1, 2, 3))
                       + oproj_units(0, (4, 5, 6, 8, 10, 12, 13, 14)),
                    7: oproj_units(1, (1, 3, 5, 7, 9, 11, 13, 14)),
                }
                _bg_done = [0]
                prev = None
                for r, (qt, hp) in enumerate(ROUNDS):
                    e2s = emit_round(qt, hp, prev, BG[r])
                    prev = (qt, hp, e2s)

                # ---- tail: PV of last round interleaved with oproj(2)
                # (its ot3 half is written by pv_norm(2,1) at the end of
                # round 7).  Then oproj(3) overlaps the final norm: the m0
                # half-accumulations run on the now-free ps2/pp banks
                # DURING the norm chain (also keeping the PE out of its
                # low p-state), and the m1 halves + copies follow.
                pqt, php, pe2s = prev
                pva = pspv.tile([DK + 1, 512], fp32, tag="pv")
                pvb = pspv.tile([DK + 1, 512], fp32, tag="pv")
                for kt in range(NKT):
                    pv_kt(php, pe2s, kt, pva, pvb)
                    if kt % 2 == 1:
                        oproj_unit(2, kt // 2, on_act=(kt % 4 == 1),
                                   dma_eng=nc.scalar)
                q3s = slice(pqt * 512, (pqt + 1) * 512)
                sr, iv, ib = [], [], []
                for h, pv in ((0, pva), (1, pvb)):
                    s = pnrm.tile([1, 512], fp32, tag="srow", name=f"sr{h}")
                    nc.vector.tensor_copy(s[:], pv[DK:DK + 1, :])
                    sr.append(s)
                for h in range(2):
                    v = pnrm.tile([1, 512], fp32, tag="inv", name=f"iv{h}")
                    nc.vector.reciprocal_approx_fast(v[:], sr[h][:])
                    iv.append(v)
                _po3[3] = pout.tile([128, NDT, 512], fp16, tag="po",
                                    name="po3t")
                t_ps = []
                for jt in range(6):   # m0 halves during the norm chain
                    if jt < 4:
                        if jt % 2 == 0:
                            s2t = ps2.tile([128, 1024], fp32, tag="s2",
                                           name=f"os{jt}")
                        ps = s2t[:, (jt % 2) * 512:(jt % 2) * 512 + 512]
                    else:
                        ps = pp.tile([128, 512], fp32, tag="acc",
                                     name=f"op{jt}")[:]
                    nc.tensor.matmul(
                        ps, wo3[:, 0, jt * 128:(jt + 1) * 128],
                        ot3[:, 0, q3s], start=True, stop=False)
                    t_ps.append(ps)
                for h, pv in ((0, pva), (1, pvb)):
                    b = pnrm.tile([64, 512], fp32, tag="invb",
                                  name=f"ib{h}")
                    nc.gpsimd.partition_broadcast(b[:], iv[h][:])
                    nc.vector.tensor_tensor(
                        ot3[h * 64:(h + 1) * 64, php, q3s],
                        pv[0:DK, :], b[:], mybir.AluOpType.mult)
                def _po_dma(jt):
                    nc.scalar.dma_start(
                        out_d[jt * 128:(jt + 1) * 128, q3s],
                        _po3[3][:, jt, :])

                for jt in range(6):   # m1 halves + copies
                    nc.tensor.matmul(
                        t_ps[jt], wo3[:, 1, jt * 128:(jt + 1) * 128],
                        ot3[:, 1, q3s], start=False, stop=True)
                    if jt % 2 == 0:
                        nc.scalar.copy(_po3[3][:, jt, :], t_ps[jt])
                    else:
                        nc.vector.tensor_copy(_po3[3][:, jt, :], t_ps[jt])
                    _po_dma(jt)
                for jt in (6, 7):     # full pairs on the freed pspv banks
                    ps = pspv.tile([128, 512], fp32, tag="pv",
                                   name=f"op{jt}")
                    for m in range(2):
                        nc.tensor.matmul(
                            ps[:], wo3[:, m, jt * 128:(jt + 1) * 128],
                            ot3[:, m, q3s], start=(m == 0), stop=(m == 1))
                    if jt == 6:
                        nc.scalar.copy(_po3[3][:, jt, :], ps[:])
                    else:
                        nc.vector.tensor_copy(_po3[3][:, jt, :], ps[:])
                    _po_dma(jt)

    nc.compile()
    return nc


def kernel(q, k, v, Wq, bq, Wk, bk, Wv, bv, Wo, bo, _trace=False):
    from concourse import bass_utils

    if "nc" not in _cache:
        _cache["nc"] = _build()
    nc = _cache["nc"]

    q = np.asarray(q, np.float32)
    k = np.asarray(k, np.float32)
    v = np.asarray(v, np.float32)
    Wq = np.asarray(Wq, np.float32)
    Wk = np.asarray(Wk, np.float32)
    Wv = np.asarray(Wv, np.float32)
    Wo = np.asarray(Wo, np.float32)
    bq = np.asarray(bq, np.float32)
    bk = np.asarray(bk, np.float32)
    bv = np.asarray(bv, np.float32)
    bo = np.asarray(bo, np.float32)

    f16 = np.float16

    # host-side pre-arrangement: all DMA rows contiguous per partition
    def arr_x_q(xT):    # [D,S] -> [128, NQT, NDT, 512]
        return np.ascontiguousarray(
            xT.reshape(NDT, 128, NQT, 512).transpose(1, 2, 0, 3)).astype(f16)

    def arr_x_v(xT):    # [D,S] -> [128, NKT, NDT, 128]
        return np.ascontiguousarray(
            xT.reshape(NDT, 128, NKT, 128).transpose(1, 2, 0, 3)).astype(f16)

    def arr_w(WslT):    # [D, FEAT] -> [128, NDT, FEAT]
        return np.ascontiguousarray(
            WslT.reshape(NDT, 128, FEAT).transpose(1, 0, 2)).astype(f16)

    xT = {}
    for b in range(B):
        xT[("q", b)] = arr_x_q(q[b].T)
        xT[("k", b)] = arr_x_q(k[b].T)
        xT[("v", b)] = arr_x_v(v[b].T)
    wT = {}
    for g in range(HG):
        sl = slice(g * FEAT, (g + 1) * FEAT)
        wT[("q", g)] = np.ascontiguousarray(
            Wq[sl, :].T.reshape(NDT, 128, 2, 128)
            .transpose(1, 2, 0, 3)).astype(f16)
        wT[("k", g)] = np.ascontiguousarray(
            Wk[sl, :].T.reshape(NDT, 128, 2, 128)
            .transpose(1, 2, 0, 3)).astype(f16)
        wT[("v", g)] = arr_w(Wv[sl, :].T)
        wT[("o", g)] = np.ascontiguousarray(
            Wo[:, sl].T.reshape(2, 128, D).transpose(1, 0, 2)).astype(f16)

    in_maps = []
    for c in range(N_CORES):
        b, g = divmod(c, HG)
        sl = slice(g * FEAT, (g + 1) * FEAT)
        in_maps.append({
            "xq_d": xT[("q", b)], "xk_d": xT[("k", b)], "xv_d": xT[("v", b)],
            "wq_d": wT[("q", g)], "wk_d": wT[("k", g)], "wv_d": wT[("v", g)],
            "wo_d": wT[("o", g)],
            "bq_d": np.ascontiguousarray(
                bq[sl].reshape(2, 128).T.reshape(128, 2, 1)),
            "bk_d": np.ascontiguousarray(
                bk[sl].reshape(2, 128).T.reshape(128, 2, 1)),
        })

    kwargs = {}
    if _trace:
        _install_profile_shim()
        kwargs = dict(trace=True, trace_cores=list(range(N_CORES)))
    res = bass_utils.run_bass_kernel_spmd(
        nc, in_maps, core_ids=list(range(N_CORES)), **kwargs)
    _cache["last_results"] = res

    final_bias = (Wo @ bv + bo).astype(np.float32)  # attn rows sum to 1
    out = np.empty((B, S, D), np.float32)
    for b in range(B):
        acc = res.results[b * HG]["partialT"].astype(np.float32)
        for g in range(1, HG):
            acc += res.results[b * HG + g]["partialT"].astype(np.float32)
        out[b] = acc.T + final_bias
    return out


def _install_profile_shim():
    """Provide antenv.axon_hooks so trace=True works under axon."""
    import sys
    import types

    import antenv

    if "antenv.axon_hooks" in sys.modules:
        return
    mod = types.ModuleType("antenv.axon_hooks")
    mod._hook = None
    mod.set_axon_ntff_profile_hook = lambda h: setattr(mod, "_hook", h)
    mod.get_axon_ntff_profile_hook = lambda: mod._hook
    sys.modules["antenv.axon_hooks"] = mod
    antenv.axon_hooks = mod
    try:
        from trn_agent_boot.trn_boot import _ntff_profile_via_ctypes
        mod.set_axon_ntff_profile_hook(
            _ntff_profile_via_ctypes("/opt/axon/libaxon_pjrt.so"))
    except Exception:
        pass
